# revision 1
# baseline (speedup 1.0000x reference)
"""Bidirectional simplified SSM kernel for Trainium2 (8 NeuronCores).

Math (per batch element b):
    z = x @ W_in                                  [L, DI]
    fwd:  o = z @ W_fwd; delta = sigmoid(o[:, :DI]); gate = o[:, DI:] * z
          h_t = delta_t * h_{t-1} + gate_t        (t ascending)
    bwd:  same with W_bwd, t descending
    y    = concat(h_fwd, h_bwd) @ W_out + x
    out  = LayerNorm(y) * gamma + beta

Sharding: 8 cores = 4 batches x 2 sequence halves. Each core receives a
2304-token context: its 2048 tokens plus a 128-token halo on each side
(zero padded at the sequence boundary).  delta = sigmoid(o) with
|o| <~ 0.8 so the recurrence forgets at >= factor ~0.3/step; a 128-token
warm-up reproduces the cross-half scan state to ~1e-20 relative.  No
cross-core communication needed.
"""

import os
import sys

for _p in ("/opt/trn_rl_repo", "/root/.axon_site/_ro/trn_rl_repo"):
    if os.path.isdir(_p) and _p not in sys.path:
        sys.path.insert(0, _p)

import numpy as np

import concourse.bacc as bacc
import concourse.bass as bass
import concourse.mybir as mybir
import concourse.tile as tile
from concourse.masks import make_identity

P = 128
LN_EPS = 1e-5

# full-problem constants
B, L, D, DI = 4, 4096, 2048, 256
HALO = 128
T_CORE = L // 2          # tokens owned per core
T_CTX = T_CORE + 2 * HALO
N_CORES = 8


def build_nc(t_ctx=T_CTX, d=D, di=DI, halo=HALO):
    """Build + compile the (uniform SPMD) single-core Bass program."""
    di2 = 2 * di
    nch = t_ctx // P           # context chunks
    t_scan = t_ctx - P         # tokens each direction scans over
    t_out = t_ctx - 2 * halo   # tokens with output
    kd = d // P                # K-blocks for the z GEMM
    ki = di // P               # K-blocks (channel groups) for DI
    mi2 = di2 // P             # output channel groups of the o GEMM
    ndg = d // 512             # 512-wide dout groups for the out GEMM
    oc_lo = halo // P          # first output chunk
    oc_hi = oc_lo + t_out // P # one past last output chunk
    assert t_ctx % P == 0 and d % 512 == 0 and di % P == 0

    f16 = mybir.dt.float16
    f32 = mybir.dt.float32
    AO = mybir.AluOpType
    AF = mybir.ActivationFunctionType

    nc = bacc.Bacc("TRN2", target_bir_lowering=False, debug=False)
    x_d = nc.dram_tensor("x", [t_ctx, d], f32, kind="ExternalInput").ap()
    win_d = nc.dram_tensor("W_in", [d, di], f32, kind="ExternalInput").ap()
    wf_d = nc.dram_tensor("W_fwd", [di, di2], f32, kind="ExternalInput").ap()
    wb_d = nc.dram_tensor("W_bwd", [di, di2], f32, kind="ExternalInput").ap()
    wo_d = nc.dram_tensor("W_out", [di2, d], f32, kind="ExternalInput").ap()
    y_d = nc.dram_tensor("y", [t_out, d], f32, kind="ExternalOutput").ap()

    with tile.TileContext(nc) as tc:
        with (
            tc.tile_pool(name="const", bufs=1) as cpool,
            tc.tile_pool(name="xin", bufs=1) as xpool,
            tc.tile_pool(name="xT", bufs=2) as xtpool,
            tc.tile_pool(name="zt", bufs=1) as zpool,
            tc.tile_pool(name="dg", bufs=1) as dgpool,
            tc.tile_pool(name="ych", bufs=3) as ypool,
            tc.tile_pool(name="sq", bufs=2) as sqpool,
            tc.tile_pool(name="st", bufs=6) as stpool,
            tc.tile_pool(name="mm", bufs=4, space="PSUM") as mmps,
            tc.tile_pool(name="tp", bufs=2, space="PSUM") as tpps,
        ):
            # ---- weights (cast to fp16 during DMA) ----
            w_in16 = cpool.tile([P, kd, di], f16)
            nc.gpsimd.dma_start(w_in16[:], win_d.rearrange("(ko p) e -> p ko e", p=P))
            w_f16 = cpool.tile([P, ki, di2], f16)
            nc.gpsimd.dma_start(w_f16[:], wf_d.rearrange("(ko p) e -> p ko e", p=P))
            w_b16 = cpool.tile([P, ki, di2], f16)
            nc.gpsimd.dma_start(w_b16[:], wb_d.rearrange("(ko p) e -> p ko e", p=P))
            w_o16 = cpool.tile([P, mi2, d], f16)
            nc.gpsimd.dma_start(w_o16[:], wo_d.rearrange("(ko p) e -> p ko e", p=P))
            ident = cpool.tile([P, P], f16)
            make_identity(nc, ident[:])
            eps_t = cpool.tile([P, 1], f32)
            nc.vector.memset(eps_t[:], LN_EPS)

            # ---- x load (fp16 resident) + transpose + z GEMM ----
            x16 = xpool.tile([P, nch, d], f16)
            zt16 = zpool.tile([P, ki, t_ctx], f16)
            for c in range(nch):
                nc.gpsimd.dma_start(x16[:, c, :], x_d[c * P:(c + 1) * P, :])
            for g0 in range(0, nch, 4):
                gch = min(4, nch - g0)
                gsz = gch * P
                xT = xtpool.tile([P, kd, 4 * P], f16)
                for ci in range(gch):
                    c = g0 + ci
                    pt = tpps.tile([P, kd, P], f16)
                    for kb in range(kd):
                        nc.tensor.transpose(
                            pt[:, kb, :], x16[:, c, kb * P:(kb + 1) * P], ident[:]
                        )
                    nc.vector.tensor_copy(xT[:, :, ci * P:(ci + 1) * P], pt[:])
                for m in range(ki):
                    pz = mmps.tile([P, 512], f32, tag="mm")
                    for kb in range(kd):
                        nc.tensor.matmul(
                            pz[:, :gsz],
                            w_in16[:, kb, m * P:(m + 1) * P],
                            xT[:, kb, :gsz],
                            start=(kb == 0),
                            stop=(kb == kd - 1),
                        )
                    nc.scalar.copy(zt16[:, m, g0 * P:g0 * P + gsz], pz[:, :gsz])

            # ---- per-direction: o GEMM + delta/gate + chained scans ----
            d_f = dgpool.tile([P, ki, t_scan], f16)
            g_f = dgpool.tile([P, ki, t_scan], f16)   # becomes h_fwd in place
            d_b = dgpool.tile([P, ki, t_scan], f16)
            g_b = dgpool.tile([P, ki, t_scan], f16)   # becomes h_bwd in place

            # out GEMM + residual + LayerNorm for one 128-token chunk;
            # called from inside the bwd loop as h_bwd segments complete.
            inv_d = 1.0 / d

            def out_chunk(oc):
                t0 = oc * P
                y_sb = ypool.tile([P, d], f32, name="y_sb")
                st = stpool.tile([P, 12], f32, name="st")
                for dgi in range(ndg):
                    py = mmps.tile([P, 512], f32, tag="mm")
                    dsl = slice(dgi * 512, (dgi + 1) * 512)
                    # residual folded into the accumulation: I.T @ x == x
                    mm_ops = [(ident[:], x16[:, oc, dsl])]
                    mm_ops += [(g_f[:, kb, t0:t0 + P], w_o16[:, kb, dsl])
                               for kb in range(ki)]
                    mm_ops += [(g_b[:, kb, t0 - P:t0], w_o16[:, ki + kb, dsl])
                               for kb in range(ki)]
                    for i, (lhsT, rhs) in enumerate(mm_ops):
                        nc.tensor.matmul(
                            py[:], lhsT, rhs,
                            start=(i == 0), stop=(i == len(mm_ops) - 1),
                        )
                    nc.scalar.activation(
                        y_sb[:, dsl], py[:], AF.Copy,
                        accum_out=st[:, dgi:dgi + 1],
                    )
                nc.vector.tensor_reduce(
                    st[:, 4:5], st[:, 0:ndg], mybir.AxisListType.X, AO.add
                )
                sq = sqpool.tile([P, d], f16, name="sq")
                nc.scalar.activation(
                    sq[:], y_sb[:], AF.Square, accum_out=st[:, 5:6]
                )
                # mean = st4/d ; var = st5/d - mean^2 + eps
                nc.vector.tensor_scalar(
                    st[:, 6:7], st[:, 4:5], inv_d, None, AO.mult
                )
                nc.vector.tensor_tensor(st[:, 7:8], st[:, 6:7], st[:, 6:7], AO.mult)
                nc.vector.scalar_tensor_tensor(
                    st[:, 8:9], st[:, 5:6], inv_d, st[:, 7:8], AO.mult, AO.subtract
                )
                nc.scalar.activation(st[:, 9:10], st[:, 8:9], AF.Sqrt, bias=eps_t[:])
                nc.vector.reciprocal(st[:, 10:11], st[:, 9:10])
                nc.vector.tensor_scalar(
                    y_sb[:], y_sb[:], st[:, 6:7], st[:, 10:11],
                    AO.subtract, AO.mult
                )
                nc.sync.dma_start(y_d[(oc - oc_lo) * P:(oc - oc_lo + 1) * P, :], y_sb[:])

            def direction(wtile, dt, gt, tok_off, reverse):
                segs = list(range(0, t_scan, 512))
                if reverse:
                    segs = segs[::-1]
                # all o GEMMs + sigmoids + gates first, then the scan chain
                # (+ output chunks): keeps the ACT sigmoid table resident and
                # the sqrt table load a one-time cost afterwards.
                for s0 in segs:
                    ssz = min(512, t_scan - s0)
                    zsl = slice(tok_off + s0, tok_off + s0 + ssz)
                    for m2 in range(mi2):
                        po = mmps.tile([P, 512], f32, tag="mm")
                        for kb in range(ki):
                            nc.tensor.matmul(
                                po[:, :ssz],
                                wtile[:, kb, m2 * P:(m2 + 1) * P],
                                zt16[:, kb, zsl],
                                start=(kb == 0),
                                stop=(kb == ki - 1),
                            )
                        if m2 < ki:
                            nc.scalar.activation(
                                dt[:, m2, s0:s0 + ssz], po[:, :ssz], AF.Sigmoid
                            )
                        else:
                            nc.vector.tensor_tensor(
                                gt[:, m2 - ki, s0:s0 + ssz],
                                po[:, :ssz],
                                zt16[:, m2 - ki, zsl],
                                AO.mult,
                            )
                first = True
                for s0 in segs:
                    ssz = min(512, t_scan - s0)
                    for kb in range(ki):
                        if not reverse:
                            init = 0.0 if first else gt[:, kb, s0 - 1:s0]
                            nc.vector.tensor_tensor_scan(
                                gt[:, kb, s0:s0 + ssz],
                                dt[:, kb, s0:s0 + ssz],
                                gt[:, kb, s0:s0 + ssz],
                                init,
                                AO.mult,
                                AO.add,
                            )
                        else:
                            hi = s0 + ssz
                            init = 0.0 if first else gt[:, kb, hi:hi + 1]
                            nc.vector.tensor_tensor_scan(
                                gt[:, kb, s0:s0 + ssz][:, ::-1],
                                dt[:, kb, s0:s0 + ssz][:, ::-1],
                                gt[:, kb, s0:s0 + ssz][:, ::-1],
                                init,
                                AO.mult,
                                AO.add,
                            )
                    first = False
                    if reverse:
                        # h_bwd indices [s0, t_scan) are now final; emit the
                        # output chunks whose h_bwd slice just completed.
                        lo = max(oc_lo, s0 // P + 1)
                        hi = min(oc_hi, (s0 + ssz) // P + 1)
                        for oc in range(hi - 1, lo - 1, -1):
                            out_chunk(oc)

            direction(w_f16, d_f, g_f, 0, reverse=False)
            direction(w_b16, d_b, g_b, P, reverse=True)

    nc.compile()
    return nc


_NC_CACHE = {}


def _get_nc(key=(T_CTX, D, DI, HALO)):
    if key not in _NC_CACHE:
        _NC_CACHE[key] = build_nc(*key)
    return _NC_CACHE[key]


def shard_inputs(x, W_in, W_fwd, W_bwd, W_out):
    """Full x [B, L, D] -> 8 per-core input dicts with halo-padded contexts."""
    xf = np.ascontiguousarray(x, dtype=np.float32)
    xp = np.zeros((B, L + 2 * HALO, D), np.float32)
    xp[:, HALO:HALO + L] = xf
    wmaps = {
        "W_in": np.ascontiguousarray(W_in, np.float32),
        "W_fwd": np.ascontiguousarray(W_fwd, np.float32),
        "W_bwd": np.ascontiguousarray(W_bwd, np.float32),
        "W_out": np.ascontiguousarray(W_out, np.float32),
    }
    in_maps = []
    for b in range(B):
        for h in range(2):
            shard = np.ascontiguousarray(xp[b, h * T_CORE:h * T_CORE + T_CTX])
            in_maps.append({"x": shard, **wmaps})
    return in_maps


def gather_outputs(results):
    out = np.empty((B, L, D), np.float32)
    for b in range(B):
        for h in range(2):
            out[b, h * T_CORE:(h + 1) * T_CORE] = results[b * 2 + h]["y"]
    return out


def run_on_hw(x, W_in, W_fwd, W_bwd, W_out, trace=False):
    from concourse.bass_utils import run_bass_kernel_spmd

    nc = _get_nc()
    in_maps = shard_inputs(x, W_in, W_fwd, W_bwd, W_out)
    res = run_bass_kernel_spmd(
        nc, in_maps, core_ids=list(range(N_CORES)), trace=trace
    )
    return gather_outputs(res.results), res


def kernel(x, W_in, W_fwd, W_bwd, W_out, gamma, beta):
    y, _ = run_on_hw(x, W_in, W_fwd, W_bwd, W_out)
    gamma = np.asarray(gamma, np.float32)
    beta = np.asarray(beta, np.float32)
    if not (np.all(gamma == 1.0) and np.all(beta == 0.0)):
        y = y * gamma + beta
    return y.astype(np.float32)



# revision 7
# speedup vs baseline: 1.6234x; 1.6234x over previous
"""Bidirectional simplified SSM kernel for Trainium2 (8 NeuronCores).

Math (per batch element b):
    z = x @ W_in                                  [L, DI]
    fwd:  o = z @ W_fwd; delta = sigmoid(o[:, :DI]); gate = o[:, DI:] * z
          h_t = delta_t * h_{t-1} + gate_t        (t ascending)
    bwd:  same with W_bwd, t descending
    y    = concat(h_fwd, h_bwd) @ W_out + x
    out  = LayerNorm(y) * gamma + beta

Sharding: 8 cores = 4 batches x 2 sequence halves, each with a 64-token
halo (delta ~ sigmoid(small) ~ 0.5 forgets cross-boundary state to
~1e-19 over 64 steps).  Host ships x twice: natural fp16 (residual/LN)
and pre-transposed fp8 (z GEMM rhs), plus weights pre-packed in SBUF
layout as fp8.  All GEMMs are fp8 DoubleRow matmuls (2 K-tiles per
instruction, 0.5 cycles/row); the residual is an fp16 identity matmul
in the same PSUM group.  The two recurrence directions each split their
2 independent channel groups across DVE and GPSIMD so all four scan
chains run concurrently.  LayerNorm: ACT copy+accum gives the row sum
for free, squares are split ACT/DVE for balance, normalize is a DVE
16-bit 4x tensor_scalar.
"""

import os
import sys

for _p in ("/opt/trn_rl_repo", "/root/.axon_site/_ro/trn_rl_repo"):
    if os.path.isdir(_p) and _p not in sys.path:
        sys.path.insert(0, _p)

import ml_dtypes
import numpy as np

import concourse.bacc as bacc
import concourse.bass as bass
import concourse.mybir as mybir
import concourse.tile as tile
from concourse.masks import make_identity

P = 128
LN_EPS = 1e-5

B, L, D, DI = 4, 4096, 2048, 256
HALO = 64
T_CORE = L // 2            # tokens owned per core
T_CTX = T_CORE + 2 * HALO  # context tokens incl. halo
T_SCAN = T_CORE + HALO     # tokens each direction scans over
N_CORES = 8

F8 = ml_dtypes.float8_e4m3
DR = mybir.MatmulPerfMode.DoubleRow

# interleaved so both scan directions get their first segment early
SEG_ORDER = [0, 4, 1, 3, 2]
# middle-out: middle chunks' h_fwd/h_bwd complete first
CHUNK_ORDER = [9, 10, 8, 11, 7, 12, 6, 13, 5, 14, 4, 15, 3, 2, 1, 0]
# chunks whose LN square runs on ACT instead of DVE (load balance)
ACT_SQUARE = {9, 3, 2, 1, 0}


def build_nc():
    d, di = D, DI
    kd = d // P            # 16 K-blocks for the z GEMM
    ki = di // P           # 2  channel groups of DI
    mi2 = 2 * di // P      # 4  output channel groups of the o GEMM
    ncho = T_CORE // P     # 16 owned output chunks
    segs = [(s, min(512, T_CTX - s)) for s in range(0, T_CTX, 512)]
    ssegs = [(s, min(512, T_SCAN - s)) for s in range(0, T_SCAN, 512)]
    nseg = len(segs)
    assert nseg == len(ssegs) == len(SEG_ORDER)

    f8 = mybir.dt.float8e4
    f16 = mybir.dt.float16
    f32 = mybir.dt.float32
    AO = mybir.AluOpType
    AF = mybir.ActivationFunctionType

    nc = bacc.Bacc("TRN2", target_bir_lowering=False, debug=False)
    xt_d = nc.dram_tensor("xT8", [P, kd, T_CTX], f8, kind="ExternalInput").ap()
    x_d = nc.dram_tensor("x16", [T_CORE, d], f16, kind="ExternalInput").ap()
    win_d = nc.dram_tensor("W_in8", [P, kd, di], f8, kind="ExternalInput").ap()
    wf_d = nc.dram_tensor("W_fwd8", [P, ki, 2 * di], f8, kind="ExternalInput").ap()
    wb_d = nc.dram_tensor("W_bwd8", [P, ki, 2 * di], f8, kind="ExternalInput").ap()
    wo_d = nc.dram_tensor("W_out8", [P, mi2, d], f8, kind="ExternalInput").ap()
    y_d = nc.dram_tensor("y", [T_CORE, d], f16, kind="ExternalOutput").ap()

    inv_d = 1.0 / d

    with tile.TileContext(nc) as tc:
        with (
            tc.tile_pool(name="const", bufs=1) as cpool,
            tc.tile_pool(name="xt", bufs=1) as xtpool,
            tc.tile_pool(name="xn", bufs=1) as xnpool,
            tc.tile_pool(name="z", bufs=1) as zpool,
            tc.tile_pool(name="dg", bufs=1) as dgpool,
            tc.tile_pool(name="y16", bufs=3) as ypool,
            tc.tile_pool(name="sq", bufs=2) as sqpool,
            tc.tile_pool(name="yo", bufs=2) as yopool,
            tc.tile_pool(name="st", bufs=4) as stpool,
            tc.tile_pool(name="ps", bufs=2, space="PSUM") as pspool,
        ):
            # ---- pool-issued input DMAs, priority order ----
            w_in8 = cpool.tile([P, kd, di], f8)
            w_f8 = cpool.tile([P, ki, 2 * di], f8)
            w_b8 = cpool.tile([P, ki, 2 * di], f8)
            w_o8 = cpool.tile([P, mi2, d], f8)
            xt8 = xtpool.tile([P, kd, T_CTX], f8)
            x16 = xnpool.tile([P, ncho, d], f16)

            nc.gpsimd.dma_start(w_in8[:], win_d)
            s0, ssz = segs[SEG_ORDER[0]]
            nc.gpsimd.dma_start(xt8[:, :, s0:s0 + ssz], xt_d[:, :, s0:s0 + ssz])
            nc.gpsimd.dma_start(w_f8[:], wf_d)
            nc.gpsimd.dma_start(w_b8[:], wb_d)
            for si in SEG_ORDER[1:]:
                s0, ssz = segs[si]
                nc.gpsimd.dma_start(
                    xt8[:, :, s0:s0 + ssz], xt_d[:, :, s0:s0 + ssz]
                )
            # WAW anchors so the SP-issued x16 DMAs enqueue on the DMA
            # engines behind the xT8 segments
            for q in range(4):
                nc.gpsimd.memset(x16[:, 4 * q, 0:1], 0.0)
            nc.gpsimd.dma_start(w_o8[:], wo_d)

            # x16 quads on SP (HWDGE), need-ordered for middle-out chunks
            for q in (2, 3, 1, 0):
                nc.sync.dma_start(
                    x16[:, 4 * q:4 * q + 4, :],
                    x_d[512 * q:512 * (q + 1), :].rearrange(
                        "(c p) d -> p c d", p=P
                    ),
                )

            ident = cpool.tile([P, P], f16)
            make_identity(nc, ident[:])
            eps_t = cpool.tile([P, 1], f32)
            nc.vector.memset(eps_t[:], LN_EPS)

            # ---- z GEMM (fp8 DoubleRow) + o GEMMs, seg-interleaved ----
            z8 = zpool.tile([P, ki, T_CTX], f8)
            d_f = dgpool.tile([P, ki, T_SCAN], f16)
            g_f = dgpool.tile([P, ki, T_SCAN], f16)
            h_f = dgpool.tile([P, ki, T_SCAN], f8)
            d_b = dgpool.tile([P, ki, T_SCAN], f16)
            g_b = dgpool.tile([P, ki, T_SCAN], f16)
            h_b = dgpool.tile([P, ki, T_SCAN], f8)

            def z_seg(si):
                s0, ssz = segs[si]
                pz = pspool.tile([P, 2048], f32, tag="ps", name="pz")
                for m in range(ki):
                    pv = pz[:, m * 512:m * 512 + ssz]
                    for k8 in range(kd // 2):
                        nc.tensor.matmul(
                            pv,
                            w_in8[:, 2 * k8:2 * k8 + 2, m * P:(m + 1) * P],
                            xt8[:, 2 * k8:2 * k8 + 2, s0:s0 + ssz],
                            start=(k8 == 0),
                            stop=(k8 == kd // 2 - 1),
                            perf_mode=DR,
                        )
                    # fp8 convert on ACT (early phase is sigmoid-light)
                    nc.scalar.copy(z8[:, m, s0:s0 + ssz], pv)

            # fwd kb chain on engine kb, bwd kb chain on engine 1-kb
            eng = [nc.vector, nc.gpsimd]

            def o_seg(si, reverse):
                s0, ssz = ssegs[si]
                tok_off = HALO if reverse else 0
                w8 = w_b8 if reverse else w_f8
                dt = d_b if reverse else d_f
                gt = g_b if reverse else g_f
                zsl = slice(tok_off + s0, tok_off + s0 + ssz)
                po = pspool.tile([P, 2048], f32, tag="ps", name="po")
                for m2 in range(mi2):
                    pv = po[:, m2 * 512:m2 * 512 + ssz]
                    nc.tensor.matmul(
                        pv,
                        w8[:, :, m2 * P:(m2 + 1) * P],
                        z8[:, :, zsl],
                        start=True,
                        stop=True,
                        perf_mode=DR,
                    )
                for m2 in range(ki):
                    nc.scalar.activation(
                        dt[:, m2, s0:s0 + ssz],
                        po[:, m2 * 512:m2 * 512 + ssz],
                        AF.Sigmoid,
                    )
                for kb in range(ki):
                    e = eng[1 - kb] if reverse else eng[kb]
                    e.tensor_tensor(
                        gt[:, kb, s0:s0 + ssz],
                        po[:, (ki + kb) * 512:(ki + kb) * 512 + ssz],
                        z8[:, kb, zsl],
                        AO.mult,
                    )

            def scan_seg(si, reverse):
                s0, ssz = ssegs[si]
                dt, gt, ht = (d_b, g_b, h_b) if reverse else (d_f, g_f, h_f)
                first = si == (len(ssegs) - 1 if reverse else 0)
                for kb in range(ki):
                    e = eng[1 - kb] if reverse else eng[kb]
                    if not reverse:
                        init = 0.0 if first else ht[:, kb, s0 - 1:s0]
                        e.tensor_tensor_scan(
                            ht[:, kb, s0:s0 + ssz],
                            dt[:, kb, s0:s0 + ssz],
                            gt[:, kb, s0:s0 + ssz],
                            init,
                            AO.mult,
                            AO.add,
                        )
                    else:
                        hi = s0 + ssz
                        init = 0.0 if first else ht[:, kb, hi:hi + 1]
                        e.tensor_tensor_scan(
                            ht[:, kb, s0:s0 + ssz][:, ::-1],
                            dt[:, kb, s0:s0 + ssz][:, ::-1],
                            gt[:, kb, s0:s0 + ssz][:, ::-1],
                            init,
                            AO.mult,
                            AO.add,
                        )

            # PE/consumer order: z segs interleaved with o segs as the
            # transposed input lands; fwd o ascending, bwd o descending.
            z_seg(SEG_ORDER[0])
            z_seg(SEG_ORDER[1])
            o_seg(0, reverse=False)
            o_seg(nseg - 1, reverse=True)
            scan_seg(0, reverse=False)
            scan_seg(nseg - 1, reverse=True)
            fwd_i, bwd_i = 1, nseg - 2
            for k in range(2, nseg):
                z_seg(SEG_ORDER[k])
                if k % 2 == 0:
                    o_seg(fwd_i, reverse=False)
                    scan_seg(fwd_i, reverse=False)
                    fwd_i += 1
                else:
                    o_seg(bwd_i, reverse=True)
                    scan_seg(bwd_i, reverse=True)
                    bwd_i -= 1
            while fwd_i < nseg or bwd_i >= 0:
                if fwd_i < nseg:
                    o_seg(fwd_i, reverse=False)
                    scan_seg(fwd_i, reverse=False)
                    fwd_i += 1
                if bwd_i >= 0:
                    o_seg(bwd_i, reverse=True)
                    scan_seg(bwd_i, reverse=True)
                    bwd_i -= 1

            # ---- out GEMM + residual + LayerNorm per owned chunk ----
            def out_chunk(oc):
                tb = HALO + oc * P     # context-token base of this chunk
                y16 = ypool.tile([P, d], f16, name="y16")
                st = stpool.tile([P, 12], f32, name="st")
                py = pspool.tile([P, 2048], f32, tag="ps", name="py")
                for dgi in range(4):
                    dsl = slice(dgi * 512, (dgi + 1) * 512)
                    pv = py[:, dsl]
                    # residual folded in: I.T @ x == x
                    nc.tensor.matmul(
                        pv, ident[:], x16[:, oc, dsl], start=True, stop=False
                    )
                    nc.tensor.matmul(
                        pv, h_f[:, :, tb:tb + P], w_o8[:, 0:2, dsl],
                        start=False, stop=False, perf_mode=DR,
                    )
                    nc.tensor.matmul(
                        pv, h_b[:, :, tb - HALO:tb - HALO + P],
                        w_o8[:, 2:4, dsl],
                        start=False, stop=True, perf_mode=DR,
                    )
                nc.scalar.activation(
                    y16[:], py[:], AF.Copy, accum_out=st[:, 0:1]
                )
                sq = sqpool.tile([P, d], f16, name="sq")
                if oc in ACT_SQUARE:
                    nc.scalar.activation(
                        sq[:], y16[:], AF.Square, accum_out=st[:, 2:3]
                    )
                else:
                    nc.vector.tensor_tensor_reduce(
                        sq[:], y16[:], y16[:], 1.0, 0.0,
                        AO.mult, AO.add, st[:, 2:3],
                    )
                # mean = st0/d ; var = st2/d - mean^2
                nc.gpsimd.tensor_scalar(st[:, 4:5], st[:, 0:1], inv_d, None, AO.mult)
                nc.gpsimd.tensor_tensor(st[:, 5:6], st[:, 4:5], st[:, 4:5], AO.mult)
                nc.gpsimd.scalar_tensor_tensor(
                    st[:, 6:7], st[:, 2:3], inv_d, st[:, 5:6], AO.mult, AO.subtract
                )
                nc.scalar.activation(st[:, 7:8], st[:, 6:7], AF.Sqrt, bias=eps_t[:])
                nc.vector.reciprocal(st[:, 8:9], st[:, 7:8])
                yo = yopool.tile([P, d], f16, name="yo")
                nc.vector.tensor_scalar(
                    yo[:], y16[:], st[:, 4:5], st[:, 8:9], AO.subtract, AO.mult
                )
                nc.sync.dma_start(y_d[oc * P:(oc + 1) * P, :], yo[:])

            for oc in CHUNK_ORDER:
                out_chunk(oc)

    nc.compile()
    return nc


_NC_CACHE = {}


def _get_nc():
    if "nc" not in _NC_CACHE:
        _NC_CACHE["nc"] = build_nc()
    return _NC_CACHE["nc"]


def _pack_weights(W_in, W_fwd, W_bwd, W_out):
    """Rearrange [K, M] weights into SBUF layout [128, K//128, M], cast fp8."""
    def pack(w):
        k, m = w.shape
        return np.ascontiguousarray(
            w.reshape(k // P, P, m).transpose(1, 0, 2)
        ).astype(F8)

    return {
        "W_in8": pack(np.asarray(W_in, np.float32)),
        "W_fwd8": pack(np.asarray(W_fwd, np.float32)),
        "W_bwd8": pack(np.asarray(W_bwd, np.float32)),
        "W_out8": pack(np.asarray(W_out, np.float32)),
    }


def shard_inputs(x, W_in, W_fwd, W_bwd, W_out):
    """Full x [B, L, D] -> 8 per-core input dicts."""
    x16 = np.asarray(x, np.float32).astype(np.float16)
    xpad = np.zeros((B, L + 2 * HALO, D), np.float16)
    xpad[:, HALO:HALO + L] = x16
    wmaps = _pack_weights(W_in, W_fwd, W_bwd, W_out)
    in_maps = []
    for b in range(B):
        for h in range(2):
            ctx = xpad[b, h * T_CORE:h * T_CORE + T_CTX]      # [T_CTX, D]
            xT8 = np.ascontiguousarray(
                ctx.T.reshape(D // P, P, T_CTX).transpose(1, 0, 2)
            ).astype(F8)                                       # [128, kd, T_CTX]
            xnat = np.ascontiguousarray(ctx[HALO:HALO + T_CORE])
            in_maps.append({"xT8": xT8, "x16": xnat, **wmaps})
    return in_maps


def gather_outputs(results):
    out = np.empty((B, L, D), np.float32)
    for b in range(B):
        for h in range(2):
            out[b, h * T_CORE:(h + 1) * T_CORE] = results[b * 2 + h]["y"]
    return out


def run_on_hw(x, W_in, W_fwd, W_bwd, W_out, trace=False):
    from concourse.bass_utils import run_bass_kernel_spmd

    nc = _get_nc()
    in_maps = shard_inputs(x, W_in, W_fwd, W_bwd, W_out)
    res = run_bass_kernel_spmd(
        nc, in_maps, core_ids=list(range(N_CORES)), trace=trace
    )
    return gather_outputs(res.results), res


def kernel(x, W_in, W_fwd, W_bwd, W_out, gamma, beta):
    y, _ = run_on_hw(x, W_in, W_fwd, W_bwd, W_out)
    gamma = np.asarray(gamma, np.float32)
    beta = np.asarray(beta, np.float32)
    if not (np.all(gamma == 1.0) and np.all(beta == 0.0)):
        y = y * gamma + beta
    return y.astype(np.float32)


# revision 8
# speedup vs baseline: 1.6740x; 1.0312x over previous
"""Bidirectional simplified SSM kernel for Trainium2 (8 NeuronCores).

Math (per batch element b):
    z = x @ W_in                                  [L, DI]
    fwd:  o = z @ W_fwd; delta = sigmoid(o[:, :DI]); gate = o[:, DI:] * z
          h_t = delta_t * h_{t-1} + gate_t        (t ascending)
    bwd:  same with W_bwd, t descending
    y    = concat(h_fwd, h_bwd) @ W_out + x
    out  = LayerNorm(y) * gamma + beta

Sharding: 8 cores = 4 batches x 2 sequence halves, each with a 64-token
halo (delta ~ sigmoid(small) ~ 0.5 forgets cross-boundary state to
~1e-19 over 64 steps).  Host ships x twice: natural fp16 (residual/LN)
and pre-transposed fp8 (z GEMM rhs), plus weights pre-packed in SBUF
layout as fp8.  All GEMMs are fp8 DoubleRow matmuls (2 K-tiles per
instruction, 0.5 cycles/row); the residual is an fp16 identity matmul
in the same PSUM group.  The two recurrence directions each split their
2 independent channel groups across DVE and GPSIMD so all four scan
chains run concurrently.  LayerNorm: ACT copy+accum gives the row sum
for free, squares are split ACT/DVE for balance, normalize is a DVE
16-bit 4x tensor_scalar.
"""

import os
import sys

for _p in ("/opt/trn_rl_repo", "/root/.axon_site/_ro/trn_rl_repo"):
    if os.path.isdir(_p) and _p not in sys.path:
        sys.path.insert(0, _p)

import ml_dtypes
import numpy as np

import concourse.bacc as bacc
import concourse.bass as bass
import concourse.mybir as mybir
import concourse.tile as tile
from concourse.masks import make_identity

P = 128
LN_EPS = 1e-5

B, L, D, DI = 4, 4096, 2048, 256
HALO = 64
T_CORE = L // 2            # tokens owned per core
T_CTX = T_CORE + 2 * HALO  # context tokens incl. halo
T_SCAN = T_CORE + HALO     # tokens each direction scans over
N_CORES = 8

F8 = ml_dtypes.float8_e4m3
DR = mybir.MatmulPerfMode.DoubleRow

# interleaved so both scan directions get their first segment early
SEG_ORDER = [0, 4, 1, 3, 2]
# middle-out: middle chunks' h_fwd/h_bwd complete first
CHUNK_ORDER = [9, 10, 8, 11, 7, 12, 6, 13, 5, 14, 4, 15, 3, 2, 1, 0]
# chunks whose LN square runs on ACT instead of DVE (load balance)
ACT_SQUARE = {9, 3, 2, 1, 0}


def build_nc():
    d, di = D, DI
    kd = d // P            # 16 K-blocks for the z GEMM
    ki = di // P           # 2  channel groups of DI
    mi2 = 2 * di // P      # 4  output channel groups of the o GEMM
    ncho = T_CORE // P     # 16 owned output chunks
    segs = [(s, min(512, T_CTX - s)) for s in range(0, T_CTX, 512)]
    ssegs = [(s, min(512, T_SCAN - s)) for s in range(0, T_SCAN, 512)]
    nseg = len(segs)
    assert nseg == len(ssegs) == len(SEG_ORDER)

    f8 = mybir.dt.float8e4
    f16 = mybir.dt.float16
    f32 = mybir.dt.float32
    AO = mybir.AluOpType
    AF = mybir.ActivationFunctionType

    nc = bacc.Bacc("TRN2", target_bir_lowering=False, debug=False)
    xt_d = nc.dram_tensor("xT8", [P, kd, T_CTX], f8, kind="ExternalInput").ap()
    x_d = nc.dram_tensor("x16", [T_CORE, d], f16, kind="ExternalInput").ap()
    win_d = nc.dram_tensor("W_in8", [P, kd, di], f8, kind="ExternalInput").ap()
    wf_d = nc.dram_tensor("W_fwd8", [P, ki, 2 * di], f8, kind="ExternalInput").ap()
    wb_d = nc.dram_tensor("W_bwd8", [P, ki, 2 * di], f8, kind="ExternalInput").ap()
    wo_d = nc.dram_tensor("W_out8", [P, mi2, d], f8, kind="ExternalInput").ap()
    y_d = nc.dram_tensor("y", [T_CORE, d], f16, kind="ExternalOutput").ap()

    inv_d = 1.0 / d

    with tile.TileContext(nc) as tc:
        with (
            tc.tile_pool(name="const", bufs=1) as cpool,
            tc.tile_pool(name="xt", bufs=1) as xtpool,
            tc.tile_pool(name="xn", bufs=1) as xnpool,
            tc.tile_pool(name="z", bufs=1) as zpool,
            tc.tile_pool(name="dg", bufs=1) as dgpool,
            tc.tile_pool(name="y16", bufs=3) as ypool,
            tc.tile_pool(name="sq", bufs=2) as sqpool,
            tc.tile_pool(name="yo", bufs=2) as yopool,
            tc.tile_pool(name="st", bufs=4) as stpool,
            tc.tile_pool(name="ps", bufs=2, space="PSUM") as pspool,
        ):
            # ---- pool-issued input DMAs, priority order ----
            w_in8 = cpool.tile([P, kd, di], f8)
            w_f8 = cpool.tile([P, ki, 2 * di], f8)
            w_b8 = cpool.tile([P, ki, 2 * di], f8)
            w_o8 = cpool.tile([P, mi2, d], f8)
            xt8 = xtpool.tile([P, kd, T_CTX], f8)
            x16 = xnpool.tile([P, ncho, d], f16)

            nc.gpsimd.dma_start(w_in8[:], win_d)
            s0, ssz = segs[SEG_ORDER[0]]
            nc.gpsimd.dma_start(xt8[:, :, s0:s0 + ssz], xt_d[:, :, s0:s0 + ssz])
            nc.gpsimd.dma_start(w_f8[:], wf_d)
            nc.gpsimd.dma_start(w_b8[:], wb_d)
            for si in SEG_ORDER[1:]:
                s0, ssz = segs[si]
                nc.gpsimd.dma_start(
                    xt8[:, :, s0:s0 + ssz], xt_d[:, :, s0:s0 + ssz]
                )
            # WAW anchors so the SP-issued x16 DMAs enqueue on the DMA
            # engines behind the xT8 segments
            for q in range(4):
                nc.gpsimd.memset(x16[:, 4 * q, 0:1], 0.0)
            nc.gpsimd.dma_start(w_o8[:], wo_d)

            # x16 quads on SP (HWDGE), need-ordered for middle-out chunks
            for q in (2, 3, 1, 0):
                nc.sync.dma_start(
                    x16[:, 4 * q:4 * q + 4, :],
                    x_d[512 * q:512 * (q + 1), :].rearrange(
                        "(c p) d -> p c d", p=P
                    ),
                )

            ident = cpool.tile([P, P], f16)
            make_identity(nc, ident[:])
            eps_t = cpool.tile([P, 1], f32)
            nc.vector.memset(eps_t[:], LN_EPS)

            # ---- z GEMM (fp8 DoubleRow) + o GEMMs, seg-interleaved ----
            z8 = zpool.tile([P, ki, T_CTX], f8)
            d_f = dgpool.tile([P, ki, T_SCAN], f16)
            g_f = dgpool.tile([P, ki, T_SCAN], f16)
            h_f = dgpool.tile([P, ki, T_SCAN], f8)
            d_b = dgpool.tile([P, ki, T_SCAN], f16)
            g_b = dgpool.tile([P, ki, T_SCAN], f16)
            h_b = dgpool.tile([P, ki, T_SCAN], f8)

            def z_seg(si):
                s0, ssz = segs[si]
                pz = pspool.tile([P, 2048], f32, tag="ps", name="pz")
                for m in range(ki):
                    pv = pz[:, m * 512:m * 512 + ssz]
                    for k8 in range(kd // 2):
                        nc.tensor.matmul(
                            pv,
                            w_in8[:, 2 * k8:2 * k8 + 2, m * P:(m + 1) * P],
                            xt8[:, 2 * k8:2 * k8 + 2, s0:s0 + ssz],
                            start=(k8 == 0),
                            stop=(k8 == kd // 2 - 1),
                            perf_mode=DR,
                        )
                    # fp8 convert on ACT (early phase is sigmoid-light)
                    nc.scalar.copy(z8[:, m, s0:s0 + ssz], pv)

            # fwd kb chain on engine kb, bwd kb chain on engine 1-kb
            eng = [nc.vector, nc.gpsimd]

            def o_seg(si, reverse):
                s0, ssz = ssegs[si]
                tok_off = HALO if reverse else 0
                w8 = w_b8 if reverse else w_f8
                dt = d_b if reverse else d_f
                gt = g_b if reverse else g_f
                zsl = slice(tok_off + s0, tok_off + s0 + ssz)
                po = pspool.tile([P, 2048], f32, tag="ps", name="po")
                for m2 in range(mi2):
                    pv = po[:, m2 * 512:m2 * 512 + ssz]
                    nc.tensor.matmul(
                        pv,
                        w8[:, :, m2 * P:(m2 + 1) * P],
                        z8[:, :, zsl],
                        start=True,
                        stop=True,
                        perf_mode=DR,
                    )
                for m2 in range(ki):
                    nc.scalar.activation(
                        dt[:, m2, s0:s0 + ssz],
                        po[:, m2 * 512:m2 * 512 + ssz],
                        AF.Sigmoid,
                    )
                for kb in range(ki):
                    e = eng[1 - kb] if reverse else eng[kb]
                    e.tensor_tensor(
                        gt[:, kb, s0:s0 + ssz],
                        po[:, (ki + kb) * 512:(ki + kb) * 512 + ssz],
                        z8[:, kb, zsl],
                        AO.mult,
                    )

            def scan_seg(si, reverse):
                s0, ssz = ssegs[si]
                dt, gt, ht = (d_b, g_b, h_b) if reverse else (d_f, g_f, h_f)
                first = si == (len(ssegs) - 1 if reverse else 0)
                for kb in range(ki):
                    e = eng[1 - kb] if reverse else eng[kb]
                    if not reverse:
                        init = 0.0 if first else ht[:, kb, s0 - 1:s0]
                        e.tensor_tensor_scan(
                            ht[:, kb, s0:s0 + ssz],
                            dt[:, kb, s0:s0 + ssz],
                            gt[:, kb, s0:s0 + ssz],
                            init,
                            AO.mult,
                            AO.add,
                        )
                    else:
                        hi = s0 + ssz
                        init = 0.0 if first else ht[:, kb, hi:hi + 1]
                        e.tensor_tensor_scan(
                            ht[:, kb, s0:s0 + ssz][:, ::-1],
                            dt[:, kb, s0:s0 + ssz][:, ::-1],
                            gt[:, kb, s0:s0 + ssz][:, ::-1],
                            init,
                            AO.mult,
                            AO.add,
                        )

            # PE/consumer order: z segs interleaved with o segs as the
            # transposed input lands; fwd o ascending, bwd o descending.
            z_seg(SEG_ORDER[0])
            z_seg(SEG_ORDER[1])
            o_seg(0, reverse=False)
            o_seg(nseg - 1, reverse=True)
            scan_seg(0, reverse=False)
            scan_seg(nseg - 1, reverse=True)
            fwd_i, bwd_i = 1, nseg - 2
            for k in range(2, nseg):
                z_seg(SEG_ORDER[k])
                if k % 2 == 0:
                    o_seg(fwd_i, reverse=False)
                    scan_seg(fwd_i, reverse=False)
                    fwd_i += 1
                else:
                    o_seg(bwd_i, reverse=True)
                    scan_seg(bwd_i, reverse=True)
                    bwd_i -= 1
            while fwd_i < nseg or bwd_i >= 0:
                if fwd_i < nseg:
                    o_seg(fwd_i, reverse=False)
                    scan_seg(fwd_i, reverse=False)
                    fwd_i += 1
                if bwd_i >= 0:
                    o_seg(bwd_i, reverse=True)
                    scan_seg(bwd_i, reverse=True)
                    bwd_i -= 1

            # ---- out GEMM + residual + LayerNorm per owned chunk ----
            # Two emission stages with a 1-chunk lag so the in-order ACT/DVE
            # queues never head-of-line block on the cross-engine stat chain.
            live = {}

            def chunk_a(oc):
                tb = HALO + oc * P     # context-token base of this chunk
                y16 = ypool.tile([P, d], f16, name="y16")
                st = stpool.tile([P, 12], f32, name="st")
                py = pspool.tile([P, 2048], f32, tag="ps", name="py")
                for dgi in range(4):
                    dsl = slice(dgi * 512, (dgi + 1) * 512)
                    pv = py[:, dsl]
                    # residual folded in: I.T @ x == x
                    nc.tensor.matmul(
                        pv, ident[:], x16[:, oc, dsl], start=True, stop=False
                    )
                    nc.tensor.matmul(
                        pv, h_f[:, :, tb:tb + P], w_o8[:, 0:2, dsl],
                        start=False, stop=False, perf_mode=DR,
                    )
                    nc.tensor.matmul(
                        pv, h_b[:, :, tb - HALO:tb - HALO + P],
                        w_o8[:, 2:4, dsl],
                        start=False, stop=True, perf_mode=DR,
                    )
                nc.scalar.activation(
                    y16[:], py[:], AF.Copy, accum_out=st[:, 0:1]
                )
                sq = sqpool.tile([P, d], f16, name="sq")
                if oc in ACT_SQUARE:
                    nc.scalar.activation(
                        sq[:], y16[:], AF.Square, accum_out=st[:, 2:3]
                    )
                else:
                    nc.vector.tensor_tensor_reduce(
                        sq[:], y16[:], y16[:], 1.0, 0.0,
                        AO.mult, AO.add, st[:, 2:3],
                    )
                # mean = st0/d ; var = st2/d - mean^2
                nc.gpsimd.tensor_scalar(st[:, 4:5], st[:, 0:1], inv_d, None, AO.mult)
                nc.gpsimd.tensor_tensor(st[:, 5:6], st[:, 4:5], st[:, 4:5], AO.mult)
                nc.gpsimd.scalar_tensor_tensor(
                    st[:, 6:7], st[:, 2:3], inv_d, st[:, 5:6], AO.mult, AO.subtract
                )
                live[oc] = (y16, st)

            def chunk_b(oc):
                y16, st = live.pop(oc)
                nc.scalar.activation(st[:, 7:8], st[:, 6:7], AF.Sqrt, bias=eps_t[:])
                yo = yopool.tile([P, d], f16, name="yo")
                nc.vector.tensor_scalar(
                    yo[:], y16[:], st[:, 4:5], st[:, 7:8], AO.subtract, AO.divide
                )
                nc.sync.dma_start(y_d[oc * P:(oc + 1) * P, :], yo[:])

            for idx, oc in enumerate(CHUNK_ORDER):
                chunk_a(oc)
                if idx >= 1:
                    chunk_b(CHUNK_ORDER[idx - 1])
            chunk_b(CHUNK_ORDER[-1])

    nc.compile()
    return nc


_NC_CACHE = {}


def _get_nc():
    if "nc" not in _NC_CACHE:
        _NC_CACHE["nc"] = build_nc()
    return _NC_CACHE["nc"]


def _pack_weights(W_in, W_fwd, W_bwd, W_out):
    """Rearrange [K, M] weights into SBUF layout [128, K//128, M], cast fp8."""
    def pack(w):
        k, m = w.shape
        return np.ascontiguousarray(
            w.reshape(k // P, P, m).transpose(1, 0, 2)
        ).astype(F8)

    return {
        "W_in8": pack(np.asarray(W_in, np.float32)),
        "W_fwd8": pack(np.asarray(W_fwd, np.float32)),
        "W_bwd8": pack(np.asarray(W_bwd, np.float32)),
        "W_out8": pack(np.asarray(W_out, np.float32)),
    }


def shard_inputs(x, W_in, W_fwd, W_bwd, W_out):
    """Full x [B, L, D] -> 8 per-core input dicts."""
    x16 = np.asarray(x, np.float32).astype(np.float16)
    xpad = np.zeros((B, L + 2 * HALO, D), np.float16)
    xpad[:, HALO:HALO + L] = x16
    wmaps = _pack_weights(W_in, W_fwd, W_bwd, W_out)
    in_maps = []
    for b in range(B):
        for h in range(2):
            ctx = xpad[b, h * T_CORE:h * T_CORE + T_CTX]      # [T_CTX, D]
            xT8 = np.ascontiguousarray(
                ctx.T.reshape(D // P, P, T_CTX).transpose(1, 0, 2)
            ).astype(F8)                                       # [128, kd, T_CTX]
            xnat = np.ascontiguousarray(ctx[HALO:HALO + T_CORE])
            in_maps.append({"xT8": xT8, "x16": xnat, **wmaps})
    return in_maps


def gather_outputs(results):
    out = np.empty((B, L, D), np.float32)
    for b in range(B):
        for h in range(2):
            out[b, h * T_CORE:(h + 1) * T_CORE] = results[b * 2 + h]["y"]
    return out


def run_on_hw(x, W_in, W_fwd, W_bwd, W_out, trace=False):
    from concourse.bass_utils import run_bass_kernel_spmd

    nc = _get_nc()
    in_maps = shard_inputs(x, W_in, W_fwd, W_bwd, W_out)
    res = run_bass_kernel_spmd(
        nc, in_maps, core_ids=list(range(N_CORES)), trace=trace
    )
    return gather_outputs(res.results), res


def kernel(x, W_in, W_fwd, W_bwd, W_out, gamma, beta):
    y, _ = run_on_hw(x, W_in, W_fwd, W_bwd, W_out)
    gamma = np.asarray(gamma, np.float32)
    beta = np.asarray(beta, np.float32)
    if not (np.all(gamma == 1.0) and np.all(beta == 0.0)):
        y = y * gamma + beta
    return y.astype(np.float32)
